# revision 5
# baseline (speedup 1.0000x reference)
"""Trainium2 Bass kernel for GraphScaledDotProductAttention.

Full shapes: q,k,v [4,8,2048,64] f32; bias, graph_attn_bias [4,8,2048,2048] f32.
  scale = 1/8
  s = (q*scale) @ k^T + bias - 0.5 * graph_attn_bias^2     (gauss_denom = 1.0)
  weights = softmax(s, axis=-1)
  graph_out = weights @ v
Returns (graph_out, weights).

Sharding: flatten (B=4,H=8) -> 32 head-pairs, 4 per core across 8 cores.
No cross-core communication. Per core the kernel is DMA-bound:
~200MB HBM traffic (bias+gbias in, weights out) at ~358 GB/s/core.
"""

import os
import sys

for _p in ("/opt/trn_rl_repo", "/opt/trn_rl_repo/concourse"):
    if os.path.isdir(_p) and _p not in sys.path:
        sys.path.insert(0, _p)

import numpy as np

import concourse.bass as bass
from concourse import bacc
import concourse.tile as tile
from concourse import mybir
from concourse.bass_utils import run_bass_kernel_spmd
from concourse.masks import make_identity

F32 = mybir.dt.float32
ALU = mybir.AluOpType
ACTF = mybir.ActivationFunctionType

B, H, S, D = 4, 8, 2048, 64
NCORES = 8
PAIRS_PER_CORE = (B * H) // NCORES  # 4
P = 128                  # partition tile (q rows per tile)
NQT = S // P             # 16 q-tiles per pair
NKC = S // P             # 16 k-chunks of 128
SCALE = 1.0 / np.sqrt(np.float32(D))   # 0.125
SQRT_HALF = float(np.sqrt(0.5))        # folds -0.5*g^2 via Square(g*sqrt(.5))
BIG = 3.0e38

last_results = None  # set per run; test.py reads exec_time_ns from here


def _build_program():
    nc = bacc.Bacc()
    q_d = nc.declare_dram_parameter("q", [PAIRS_PER_CORE, S, D], F32, isOutput=False)
    k_d = nc.declare_dram_parameter("k", [PAIRS_PER_CORE, S, D], F32, isOutput=False)
    v_d = nc.declare_dram_parameter("v", [PAIRS_PER_CORE, S, D], F32, isOutput=False)
    bias_d = nc.declare_dram_parameter(
        "bias", [PAIRS_PER_CORE, S, S], F32, isOutput=False
    )
    g_d = nc.declare_dram_parameter(
        "gbias", [PAIRS_PER_CORE, S, S], F32, isOutput=False
    )
    out_d = nc.declare_dram_parameter("out", [PAIRS_PER_CORE, S, D], F32, isOutput=True)
    w_d = nc.declare_dram_parameter("weights", [PAIRS_PER_CORE, S, S], F32, isOutput=True)

    with tile.TileContext(nc) as tc:
        with (
            tc.tile_pool(name="const", bufs=1) as const_pool,
            tc.tile_pool(name="kv", bufs=2) as kv_pool,
            tc.tile_pool(name="io", bufs=3) as io_pool,
            tc.tile_pool(name="work", bufs=2) as work_pool,
            tc.tile_pool(name="wts", bufs=4) as wts_pool,
            tc.tile_pool(name="small", bufs=4) as small_pool,
            tc.tile_pool(name="scores_pp", bufs=1, space="PSUM") as scores_pp,
            tc.tile_pool(name="trans_pp", bufs=2, space="PSUM") as trans_pp,
            tc.tile_pool(name="out_pp", bufs=2, space="PSUM") as out_pp,
        ):
            identity = const_pool.tile([P, P], F32)
            make_identity(nc, identity)

            for pr in range(PAIRS_PER_CORE):
                # --- per-pair: kT [64, 2048] via PE transposes; v as [128,16,64]
                kT = kv_pool.tile([D, S], F32)
                for cc in range(NKC // 4):
                    ktp = trans_pp.tile([P, 512], F32, tag="tp")
                    for u in range(4):
                        c = cc * 4 + u
                        kchunk = small_pool.tile([P, D], F32)
                        nc.sync.dma_start(
                            out=kchunk, in_=k_d[pr, c * P : (c + 1) * P, :]
                        )
                        nc.tensor.transpose(
                            out=ktp[:D, u * P : (u + 1) * P],
                            in_=kchunk,
                            identity=identity,
                        )
                    nc.vector.tensor_copy(
                        out=kT[:, cc * 512 : (cc + 1) * 512], in_=ktp[:D, :]
                    )
                v_sb = kv_pool.tile([P, NKC, D], F32)
                nc.sync.dma_start(
                    out=v_sb, in_=v_d[pr].rearrange("(c p) d -> p c d", p=P)
                )

                for qt in range(NQT):
                    # --- load q tile, transpose, fold in 1/sqrt(D)
                    q_nat = small_pool.tile([P, D], F32)
                    nc.sync.dma_start(out=q_nat, in_=q_d[pr, qt * P : (qt + 1) * P, :])
                    qtp = trans_pp.tile([P, 512], F32, tag="tp")
                    nc.tensor.transpose(out=qtp[:D, :P], in_=q_nat, identity=identity)
                    qT = small_pool.tile([D, P], F32)
                    nc.scalar.activation(
                        out=qT, in_=qtp[:D, :P], func=ACTF.Copy, scale=float(SCALE)
                    )

                    # --- stream in bias and gbias tiles
                    bias_t = io_pool.tile([P, S], F32)
                    nc.sync.dma_start(
                        out=bias_t, in_=bias_d[pr, qt * P : (qt + 1) * P, :]
                    )
                    g_t = io_pool.tile([P, S], F32)
                    nc.sync.dma_start(out=g_t, in_=g_d[pr, qt * P : (qt + 1) * P, :])
                    gsq = work_pool.tile([P, S], F32)
                    nc.scalar.activation(
                        out=gsq, in_=g_t, func=ACTF.Square, scale=SQRT_HALF
                    )

                    # --- scores = qT.T @ kT  (4 matmuls of N=512 into one psum tile)
                    scores = scores_pp.tile([P, S], F32)
                    for j in range(4):
                        nc.tensor.matmul(
                            out=scores[:, j * 512 : (j + 1) * 512],
                            lhsT=qT,
                            rhs=kT[:, j * 512 : (j + 1) * 512],
                            start=True,
                            stop=True,
                        )

                    # --- softmax: s = scores + (bias - 0.5 g^2)
                    # bias - gsq on GpSimd (frees DVE, which carries the
                    # psum-reading add + reduce + wT copies)
                    t_bg = work_pool.tile([P, S], F32)
                    nc.gpsimd.tensor_tensor(
                        out=t_bg, in0=bias_t, in1=gsq, op=ALU.subtract
                    )
                    s_t = work_pool.tile([P, S], F32)
                    nc.vector.tensor_tensor(
                        out=s_t, in0=scores, in1=t_bg, op=ALU.add
                    )
                    negmax = small_pool.tile([P, 1], F32)
                    nc.vector.tensor_reduce(
                        out=negmax,
                        in_=s_t,
                        axis=mybir.AxisListType.X,
                        op=ALU.max,
                        negate=True,
                    )
                    # p = exp(s - max) ; rowsum accumulated by ACT
                    p_t = work_pool.tile([P, S], F32)
                    rowsum = small_pool.tile([P, 1], F32)
                    nc.scalar.activation(
                        out=p_t,
                        in_=s_t,
                        func=ACTF.Exp,
                        bias=negmax,
                        scale=1.0,
                        accum_out=rowsum,
                    )
                    recip = small_pool.tile([P, 1], F32)
                    nc.vector.reciprocal(recip, rowsum)

                    # --- normalized weights out
                    w_t = io_pool.tile([P, S], F32)
                    nc.scalar.activation(
                        out=w_t, in_=p_t, func=ACTF.Copy, scale=recip
                    )
                    nc.sync.dma_start(
                        out=w_d[pr, qt * P : (qt + 1) * P, :], in_=w_t
                    )

                    # --- out = (p @ v) * recip : transpose p chunks, accumulate
                    out_ps = out_pp.tile([P, D], F32)
                    for cc in range(NKC // 4):
                        ptp = trans_pp.tile([P, 512], F32, tag="tp")
                        for u in range(4):
                            c = cc * 4 + u
                            nc.tensor.transpose(
                                out=ptp[:, u * P : (u + 1) * P],
                                in_=p_t[:, c * P : (c + 1) * P],
                                identity=identity,
                            )
                        wTs = wts_pool.tile([P, 512], F32)
                        nc.vector.tensor_copy(out=wTs, in_=ptp)
                        for u in range(4):
                            c = cc * 4 + u
                            nc.tensor.matmul(
                                out=out_ps,
                                lhsT=wTs[:, u * P : (u + 1) * P],
                                rhs=v_sb[:, c, :],
                                start=(c == 0),
                                stop=(c == NKC - 1),
                                skip_group_check=True,
                            )
                    out_sb = small_pool.tile([P, D], F32)
                    nc.scalar.activation(
                        out=out_sb, in_=out_ps, func=ACTF.Copy, scale=recip
                    )
                    nc.sync.dma_start(
                        out=out_d[pr, qt * P : (qt + 1) * P, :], in_=out_sb
                    )
    nc.finalize()
    return nc


def kernel(q, k, v, bias, graph_attn_bias):
    global last_results
    q = np.ascontiguousarray(np.asarray(q, dtype=np.float32)).reshape(B * H, S, D)
    k = np.ascontiguousarray(np.asarray(k, dtype=np.float32)).reshape(B * H, S, D)
    v = np.ascontiguousarray(np.asarray(v, dtype=np.float32)).reshape(B * H, S, D)
    bias = np.ascontiguousarray(np.asarray(bias, dtype=np.float32)).reshape(B * H, S, S)
    g = np.ascontiguousarray(np.asarray(graph_attn_bias, dtype=np.float32)).reshape(
        B * H, S, S
    )

    nc = _build_program()

    in_maps = []
    for i in range(NCORES):
        sl = slice(i * PAIRS_PER_CORE, (i + 1) * PAIRS_PER_CORE)
        in_maps.append(
            {
                "q": q[sl],
                "k": k[sl],
                "v": v[sl],
                "bias": bias[sl],
                "gbias": g[sl],
            }
        )

    trace = bool(os.environ.get("KERNEL_TRACE"))
    res = run_bass_kernel_spmd(
        nc, in_maps, list(range(NCORES)), trace=trace
    )
    last_results = res

    out = np.concatenate([res.results[i]["out"] for i in range(NCORES)], axis=0)
    weights = np.concatenate(
        [res.results[i]["weights"] for i in range(NCORES)], axis=0
    )
    return (
        out.reshape(B, H, S, D),
        weights.reshape(B, H, S, S),
    )
